# revision 7
# baseline (speedup 1.0000x reference)
"""Multi-head attention Bass kernel for Trainium2, 8 NeuronCores.

Problem: B=2, R=16, C=512, E=1024, H=16 heads, D=64.
  q,k,v = x @ w{q,k,v} + b{q,k,v}  (per-head attention)  out = ctx @ wo + bo

Sharding: pure data parallel over the B*R = 32 independent (batch,row)
sequences -> 4 sequences of 512 tokens per core. No collectives.

Per-core plan (all matmuls float32r: ~1.5e-4 rel err, full PE rate at N>=512):
  - PE-transpose x tiles -> xT [feat, tok] (fp32, exact)
  - Q^T, K^T produced transposed:  psum[feat128, tok512] = wq_chunk.T @ xT
  - V produced natural:            psum[tok128, feat512] = xT_chunk.T @ wv
  - S^T[kj,qi] per head = (K^T chunk).T @ Q^T ; heads packed in K=64
    row-tile pairs (tile_position (0,0)/(64,0), separate psum banks)
  - P^T = exp(S^T/8) on ACT (no max subtraction: |logits| < ~3 for this
    data distribution, exp is safe in fp32)
  - ctx^T[d,qi] per head: 4-chunk psum accumulation of V_chunk.T @ P^T
  - l (softmax denom) broadcast over partitions free via all-ones
    stationary matmul; 1/l on DVE in fp32; normalize with tensor_tensor
  - ctx^T halves DMA'd to a DRAM bounce buffer (gives the odd head its
    partition shift for free), read back as O-projection stationaries
  - O = ctxT_chunk.T @ wo + bo -> DMA out

float32r hardware constraints honored: every producer of a matmul input
writes dtype float32r; dst base partition 0; no split-K accumulation
groups; no col-tiling.
"""

import numpy as np

import concourse.bacc as bacc
import concourse.mybir as mybir
import concourse.tile as tile
from concourse import bass_utils
from concourse.masks import make_identity

F32 = mybir.dt.float32
F32R = mybir.dt.float32r

B, R, C, E, H = 2, 16, 512, 1024, 16
D = E // H            # 64
NCORES = 8
SEQS = (B * R) // NCORES   # 4 sequences per core
TCH = C // 128             # 4 token chunks per sequence
KCH = E // 128             # 8 contraction chunks
NCH = E // 512             # 2 output column chunks
PAIRS = H // 2             # 8 head pairs
SCALE = 1.0 / np.sqrt(D)   # folded into exp


def build_nc():
    nc = bacc.Bacc("TRN2", debug=False, num_devices=NCORES)

    xs_d = nc.dram_tensor("xs", [SEQS * C, E], F32, kind="ExternalInput").ap()
    w_d = {}
    for w in ("wq", "wk", "wv", "wo"):
        w_d[w] = nc.dram_tensor(w, [E, E], F32R, kind="ExternalInput").ap()
    b_d = {}
    for b in ("bq", "bk", "bv", "bo"):
        b_d[b] = nc.dram_tensor(b, [E], F32, kind="ExternalInput").ap()
    os_d = nc.dram_tensor("os", [SEQS * C, E], F32, kind="ExternalOutput").ap()

    with tile.TileContext(nc) as tc:
        with (
            tc.tile_pool(name="consts", bufs=1) as cpool,
            tc.tile_pool(name="wpool", bufs=3) as wpool,
            tc.tile_pool(name="xin", bufs=4) as xinp,
            tc.tile_pool(name="xT", bufs=8) as xTp,
            tc.tile_pool(name="vsb", bufs=4) as vp,
            tc.tile_pool(name="qk", bufs=4) as qkp,
            tc.tile_pool(name="pt", bufs=8) as ptp,
            tc.tile_pool(name="rbb", bufs=2) as rbbp,
            tc.tile_pool(name="stg", bufs=3) as stgp,
            tc.tile_pool(name="osb", bufs=2) as osbp,
            tc.tile_pool(name="ctxb", bufs=8) as ctxbp,
            tc.tile_pool(name="dram", bufs=1, space="DRAM") as dpool,
            tc.tile_pool(name="ps_pj", bufs=2, space="PSUM") as ps_pj,
            tc.tile_pool(name="ps_s", bufs=2, space="PSUM") as ps_s,
            tc.tile_pool(name="ps_c", bufs=2, space="PSUM") as ps_c,
            tc.tile_pool(name="ps_l", bufs=2, space="PSUM") as ps_l,
        ):
            # ---------------- constants ----------------
            ident = cpool.tile([128, 128], F32, name="ident")
            make_identity(nc, ident[:])
            ones_f = cpool.tile([128, 128], F32, name="ones_f")
            nc.vector.memset(ones_f[:], 1.0)
            onesr = cpool.tile([128, 128], F32R, name="onesr")
            nc.vector.tensor_copy(onesr[:], ones_f[:])

            # per-partition bias layouts: t[p, j] = b[j*128 + p]
            bqt = cpool.tile([128, KCH], F32, name="bqt")
            bkt = cpool.tile([128, KCH], F32, name="bkt")
            for name, t in (("bq", bqt), ("bk", bkt)):
                src = b_d[name].rearrange("(j p) -> p j", p=128)
                nc.sync.dma_start(t[:], src)

            # bv/bo broadcast to all 128 partitions (free-dim biases) via
            # all-ones outer product matmul
            bvb = cpool.tile([128, E], F32, name="bvb")
            bob = cpool.tile([128, E], F32, name="bob")
            # bv at partition 0, bo at partition 32 (matmul base_partition
            # must be in {0, 32, 64})
            brow = xinp.tile([33, E], F32, name="brow", tag="xin")
            nc.sync.dma_start(brow[0:1, :], b_d["bv"].rearrange("(o e) -> o e", o=1))
            nc.sync.dma_start(brow[32:33, :], b_d["bo"].rearrange("(o e) -> o e", o=1))
            browr = xinp.tile([33, E], F32R, name="browr", tag="xin")
            nc.vector.tensor_copy(browr[0:1, :], brow[0:1, :])
            nc.vector.tensor_copy(browr[32:33, :], brow[32:33, :])
            for j, dst in ((0, bvb), (32, bob)):
                for n in range(NCH):
                    pb = ps_pj.tile([128, 512], F32, name=f"pb{j}{n}", tag="pj")
                    nc.tensor.matmul(
                        pb[:], onesr[j:j + 1, :],
                        browr[j:j + 1, n * 512:(n + 1) * 512],
                        start=True, stop=True)
                    nc.vector.tensor_copy(dst[:, n * 512:(n + 1) * 512], pb[:])

            # weights: one [128, KCH*1024] tile per matrix; wo reuses a slot
            # after wq is dead (bufs=3)
            def load_w(name):
                t = wpool.tile([128, KCH * E], F32R, name=name, tag="w")
                for k in range(KCH):
                    nc.sync.dma_start(
                        t[:, k * E:(k + 1) * E], w_d[name][k * 128:(k + 1) * 128, :])
                return t

            wq_sb = load_w("wq")
            wk_sb = load_w("wk")
            wv_sb = load_w("wv")

            # ctx^T bounce buffer in DRAM
            ctxT_dram = dpool.tile([E, SEQS * C], F32R, name="ctxT_dram")

            # ---------------- phase A: projections + attention ----------
            for s in range(SEQS):
                # load x tiles [tok 128, E]
                xin = []
                for t in range(TCH):
                    xt = xinp.tile([128, E], F32, name=f"xin{s}_{t}", tag="xin")
                    nc.sync.dma_start(
                        xt[:], xs_d[s * C + t * 128: s * C + (t + 1) * 128, :])
                    xin.append(xt)

                # transpose -> xT[f] = [feat 128, tok 512] (f32r)
                xT = []
                for f in range(KCH):
                    ptr = ps_pj.tile([128, 512], F32, name=f"ptr{s}_{f}", tag="pj")
                    for t in range(TCH):
                        nc.tensor.transpose(
                            ptr[:, t * 128:(t + 1) * 128],
                            xin[t][:, f * 128:(f + 1) * 128], ident[:])
                    xf = xTp.tile([128, 512], F32R, name=f"xT{s}_{f}", tag="xT")
                    nc.vector.tensor_copy(xf[:], ptr[:])
                    xT.append(xf)

                # V projection: natural layout [tok 128, E] + bias
                vsb = []
                for t in range(TCH):
                    vt = vp.tile([128, E], F32R, name=f"v{s}_{t}", tag="v")
                    for n in range(NCH):
                        pv = ps_pj.tile([128, 512], F32, name=f"pv{s}_{t}{n}", tag="pj")
                        for k in range(KCH):
                            nc.tensor.matmul(
                                pv[:],
                                xT[k][:, t * 128:(t + 1) * 128],
                                wv_sb[:, k * E + n * 512: k * E + (n + 1) * 512],
                                start=(k == 0), stop=(k == KCH - 1))
                        nc.vector.tensor_tensor(
                            vt[:, n * 512:(n + 1) * 512], pv[:],
                            bvb[:, n * 512:(n + 1) * 512],
                            op=mybir.AluOpType.add)
                    vsb.append(vt)

                for p in range(PAIRS):
                    # Q^T / K^T for this feature pair [128 feat, 512 tok]
                    qkt = {}
                    for nm, wsb, bt in (("q", wq_sb, bqt), ("k", wk_sb, bkt)):
                        pq = ps_pj.tile([128, 512], F32, name=f"pq{nm}{s}_{p}", tag="pj")
                        for k in range(KCH):
                            nc.tensor.matmul(
                                pq[:],
                                wsb[:, k * E + p * 128: k * E + (p + 1) * 128],
                                xT[k][:],
                                start=(k == 0), stop=(k == KCH - 1))
                        qt = qkp.tile([128, 512], F32R, name=f"{nm}T{s}_{p}", tag="qk")
                        nc.vector.tensor_scalar_add(qt[:], pq[:], bt[:, p:p + 1])
                        qkt[nm] = qt
                    QT, KT = qkt["q"], qkt["k"]

                    # S^T chunks + exp -> P^T, per head (row-tiled pairs)
                    PT = [[None] * TCH for _ in range(2)]
                    for c in range(TCH):
                        pse = ps_s.tile([128, 512], F32, name=f"pse{s}{p}{c}", tag="s")
                        pso = ps_s.tile([128, 512], F32, name=f"pso{s}{p}{c}", tag="s")
                        nc.tensor.matmul(
                            pse[:], KT[0:64, c * 128:(c + 1) * 128], QT[0:64, :],
                            start=True, stop=True, tile_position=(0, 0))
                        nc.tensor.matmul(
                            pso[:], KT[64:128, c * 128:(c + 1) * 128], QT[64:128, :],
                            start=True, stop=True, tile_position=(64, 0))
                        for hh, ps_t in ((0, pse), (1, pso)):
                            pt_t = ptp.tile([128, 512], F32R,
                                            name=f"pt{s}{p}{c}{hh}", tag="pt")
                            nc.scalar.activation(
                                pt_t[:], ps_t[:],
                                mybir.ActivationFunctionType.Exp, scale=float(SCALE))
                            PT[hh][c] = pt_t

                    # ctx^T + l per head, normalize, stage out to DRAM
                    for hh in range(2):
                        h = 2 * p + hh
                        pc = ps_c.tile([64, 512], F32, name=f"pc{s}{p}{hh}", tag="c")
                        pl = ps_l.tile([64, 512], F32, name=f"pl{s}{p}{hh}", tag="l")
                        for c in range(TCH):
                            nc.tensor.matmul(
                                pc[:], vsb[c][:, h * D:(h + 1) * D], PT[hh][c][:],
                                start=(c == 0), stop=(c == TCH - 1))
                        for c in range(TCH):
                            nc.tensor.matmul(
                                pl[:], onesr[:, 0:64], PT[hh][c][:],
                                start=(c == 0), stop=(c == TCH - 1))
                        rb = rbbp.tile([64, 512], F32, name=f"rb{s}{p}{hh}", tag="rb")
                        nc.vector.reciprocal(rb[:], pl[:])
                        st = stgp.tile([64, 512], F32R, name=f"st{s}{p}{hh}", tag="st")
                        nc.vector.tensor_tensor(st[:], pc[:], rb[:],
                                                op=mybir.AluOpType.mult)
                        nc.sync.dma_start(
                            ctxT_dram[p * 128 + hh * 64: p * 128 + (hh + 1) * 64,
                                      s * C:(s + 1) * C],
                            st[:])

            # ---------------- phase B: output projection ----------------
            wo_sb = load_w("wo")
            for s in range(SEQS):
                ctxb = []
                for f in range(KCH):
                    cb = ctxbp.tile([128, 512], F32R, name=f"cb{s}_{f}", tag="cb")
                    nc.sync.dma_start(
                        cb[:], ctxT_dram[f * 128:(f + 1) * 128, s * C:(s + 1) * C])
                    ctxb.append(cb)
                for t in range(TCH):
                    for n in range(NCH):
                        po = ps_pj.tile([128, 512], F32, name=f"po{s}{t}{n}", tag="pj")
                        for k in range(KCH):
                            nc.tensor.matmul(
                                po[:],
                                ctxb[k][:, t * 128:(t + 1) * 128],
                                wo_sb[:, k * E + n * 512: k * E + (n + 1) * 512],
                                start=(k == 0), stop=(k == KCH - 1))
                        ob = osbp.tile([128, 512], F32, name=f"ob{s}{t}{n}", tag="ob")
                        nc.vector.tensor_tensor(
                            ob[:], po[:], bob[:, n * 512:(n + 1) * 512],
                            op=mybir.AluOpType.add)
                        nc.sync.dma_start(
                            os_d[s * C + t * 128: s * C + (t + 1) * 128,
                                 n * 512:(n + 1) * 512],
                            ob[:])

    nc.compile()
    return nc


_NC_CACHE = {}


def get_nc():
    if "nc" not in _NC_CACHE:
        _NC_CACHE["nc"] = build_nc()
    return _NC_CACHE["nc"]


def make_in_maps(x, wq, bq, wk, bk, wv, bv, wo, bo):
    x = np.asarray(x, dtype=np.float32)
    args = {n: np.asarray(v, dtype=np.float32)
            for n, v in (("wq", wq), ("bq", bq), ("wk", wk), ("bk", bk),
                         ("wv", wv), ("bv", bv), ("wo", wo), ("bo", bo))}
    xf = x.reshape(B * R, C, E)
    in_maps = []
    for c in range(NCORES):
        m = dict(args)
        m["xs"] = np.ascontiguousarray(
            xf[c * SEQS:(c + 1) * SEQS].reshape(SEQS * C, E))
        in_maps.append(m)
    return in_maps


def kernel(x, wq, bq, wk, bk, wv, bv, wo, bo):
    in_maps = make_in_maps(x, wq, bq, wk, bk, wv, bv, wo, bo)
    nc = get_nc()
    res = bass_utils.run_bass_kernel_spmd(
        nc, in_maps, core_ids=list(range(NCORES)))
    out = np.concatenate(
        [res.results[c]["os"].reshape(SEQS, C, E) for c in range(NCORES)], axis=0)
    return out.reshape(B, R, C, E).astype(np.float32)


# revision 8
# speedup vs baseline: 1.2534x; 1.2534x over previous
"""Multi-head attention Bass kernel for Trainium2, 8 NeuronCores.

Problem: B=2, R=16, C=512, E=1024, H=16 heads, D=64.
  q,k,v = x @ w{q,k,v} + b{q,k,v}  (per-head attention)  out = ctx @ wo + bo

Sharding: pure data parallel over the B*R = 32 independent (batch,row)
sequences -> 4 sequences of 512 tokens per core. No collectives.

Per-core plan (all matmuls float32r: ~1.5e-4 rel err, full PE rate at N>=512):
  - PE-transpose x tiles -> xT [feat, tok] (fp32, exact)
  - Q^T, K^T produced transposed:  psum[feat128, tok512] = wq_chunk.T @ xT
  - V produced natural:            psum[tok128, feat512] = xT_chunk.T @ wv
  - S^T[kj,qi] per head = (K^T chunk).T @ Q^T ; heads packed in K=64
    row-tile pairs (tile_position (0,0)/(64,0), separate psum banks)
  - P^T = exp(S^T/8) on ACT (no max subtraction: |logits| < ~3 for this
    data distribution, exp is safe in fp32)
  - ctx^T[d,qi] per head: 4-chunk psum accumulation of V_chunk.T @ P^T
  - l (softmax denom) broadcast over partitions free via all-ones
    stationary matmul; 1/l on DVE in fp32; normalize with tensor_tensor
  - ctx^T halves DMA'd to a DRAM bounce buffer (gives the odd head its
    partition shift for free), read back as O-projection stationaries
  - O = ctxT_chunk.T @ wo + bo -> DMA out

float32r hardware constraints honored: every producer of a matmul input
writes dtype float32r; dst base partition 0; no split-K accumulation
groups; no col-tiling.
"""

import numpy as np

import concourse.bacc as bacc
import concourse.mybir as mybir
import concourse.tile as tile
from concourse import bass_utils
from concourse.masks import make_identity

F32 = mybir.dt.float32
F32R = mybir.dt.float32r

B, R, C, E, H = 2, 16, 512, 1024, 16
D = E // H            # 64
NCORES = 8
SEQS = (B * R) // NCORES   # 4 sequences per core
TCH = C // 128             # 4 token chunks per sequence
KCH = E // 128             # 8 contraction chunks
NCH = E // 512             # 2 output column chunks
PAIRS = H // 2             # 8 head pairs
SCALE = 1.0 / np.sqrt(D)   # folded into exp


def build_nc():
    nc = bacc.Bacc("TRN2", debug=False, num_devices=NCORES)

    xs_d = nc.dram_tensor("xs", [SEQS * C, E], F32, kind="ExternalInput").ap()
    w_d = {}
    for w in ("wq", "wk", "wv", "wo"):
        w_d[w] = nc.dram_tensor(w, [E, E], F32R, kind="ExternalInput").ap()
    b_d = {}
    for b in ("bq", "bk", "bv", "bo"):
        b_d[b] = nc.dram_tensor(b, [E], F32, kind="ExternalInput").ap()
    os_d = nc.dram_tensor("os", [SEQS * C, E], F32, kind="ExternalOutput").ap()

    with tile.TileContext(nc) as tc:
        with (
            tc.tile_pool(name="consts", bufs=1) as cpool,
            tc.tile_pool(name="wpool", bufs=3) as wpool,
            tc.tile_pool(name="xin", bufs=4) as xinp,
            tc.tile_pool(name="xT", bufs=9) as xTp,
            tc.tile_pool(name="vsb", bufs=4) as vp,
            tc.tile_pool(name="qk", bufs=3) as qkp,
            tc.tile_pool(name="pt", bufs=12) as ptp,
            tc.tile_pool(name="nrm", bufs=3) as nrmp,
            tc.tile_pool(name="stg", bufs=3) as stgp,
            tc.tile_pool(name="osb", bufs=2) as osbp,
            tc.tile_pool(name="dram", bufs=1, space="DRAM") as dpool,
            tc.tile_pool(name="ps_pj", bufs=2, space="PSUM") as ps_pj,
            tc.tile_pool(name="ps_s", bufs=4, space="PSUM") as ps_s,
            tc.tile_pool(name="ps_c", bufs=2, space="PSUM") as ps_c,
        ):
            # ---------------- constants ----------------
            ident = cpool.tile([128, 128], F32, name="ident")
            make_identity(nc, ident[:])
            ones_f = cpool.tile([128, 128], F32, name="ones_f")
            nc.vector.memset(ones_f[:], 1.0)
            onesr = cpool.tile([128, 128], F32R, name="onesr")
            nc.vector.tensor_copy(onesr[:], ones_f[:])

            # per-partition bias layouts: t[p, j] = b[j*128 + p]
            bqt = cpool.tile([128, KCH], F32, name="bqt")
            bkt = cpool.tile([128, KCH], F32, name="bkt")
            for name, t in (("bq", bqt), ("bk", bkt)):
                src = b_d[name].rearrange("(j p) -> p j", p=128)
                nc.sync.dma_start(t[:], src)

            # bv/bo broadcast to all 128 partitions (free-dim biases) via
            # all-ones outer product matmul
            bvb = cpool.tile([128, E], F32, name="bvb")
            bob = cpool.tile([128, E], F32, name="bob")
            # bv at partition 0, bo at partition 32 (matmul base_partition
            # must be in {0, 32, 64})
            brow = xinp.tile([33, E], F32, name="brow", tag="xin")
            nc.sync.dma_start(brow[0:1, :], b_d["bv"].rearrange("(o e) -> o e", o=1))
            nc.sync.dma_start(brow[32:33, :], b_d["bo"].rearrange("(o e) -> o e", o=1))
            browr = xinp.tile([33, E], F32R, name="browr", tag="xin")
            nc.vector.tensor_copy(browr[0:1, :], brow[0:1, :])
            nc.vector.tensor_copy(browr[32:33, :], brow[32:33, :])
            for j, dst in ((0, bvb), (32, bob)):
                for n in range(NCH):
                    pb = ps_pj.tile([128, 512], F32, name=f"pb{j}{n}", tag="pj")
                    nc.tensor.matmul(
                        pb[:], onesr[j:j + 1, :],
                        browr[j:j + 1, n * 512:(n + 1) * 512],
                        start=True, stop=True)
                    nc.vector.tensor_copy(dst[:, n * 512:(n + 1) * 512], pb[:])

            # weights: one [128, KCH*1024] tile per matrix; wo reuses a slot
            # after wq is dead (bufs=3)
            def load_w(name):
                t = wpool.tile([128, KCH * E], F32R, name=name, tag="w")
                for k in range(KCH):
                    nc.sync.dma_start(
                        t[:, k * E:(k + 1) * E], w_d[name][k * 128:(k + 1) * 128, :])
                return t

            wq_sb = load_w("wq")
            wk_sb = load_w("wk")
            wv_sb = load_w("wv")

            # ctx^T bounce buffer in DRAM
            ctxT_dram = dpool.tile([E, SEQS * C], F32R, name="ctxT_dram")

            # ---------------- phase A: projections + attention ----------
            for s in range(SEQS):
                # load x tiles [tok 128, E]
                xin = []
                for t in range(TCH):
                    xt = xinp.tile([128, E], F32, name=f"xin{s}_{t}", tag="xin")
                    nc.sync.dma_start(
                        xt[:], xs_d[s * C + t * 128: s * C + (t + 1) * 128, :])
                    xin.append(xt)

                # transpose -> xT[f] = [feat 128, tok 512] (f32r)
                xT = []
                for f in range(KCH):
                    ptr = ps_pj.tile([128, 512], F32, name=f"ptr{s}_{f}", tag="pj")
                    for t in range(TCH):
                        nc.tensor.transpose(
                            ptr[:, t * 128:(t + 1) * 128],
                            xin[t][:, f * 128:(f + 1) * 128], ident[:])
                    xf = xTp.tile([128, 512], F32R, name=f"xT{s}_{f}", tag="xT")
                    nc.vector.tensor_copy(xf[:], ptr[:])
                    xT.append(xf)

                # V projection: natural layout [tok 128, 16*(64+1)] with a
                # ones column appended per head (fused softmax-denominator)
                vsb = []
                for t in range(TCH):
                    vt = vp.tile([128, H * (D + 1)], F32R, name=f"v{s}_{t}", tag="v")
                    vt3 = vt[:].rearrange("p (h dd) -> p h dd", dd=D + 1)
                    nc.vector.tensor_copy(
                        vt3[:, :, D:D + 1],
                        ones_f[:].rearrange("p (a b) -> p a b", b=1)[:, 0:H, :])
                    for n in range(NCH):
                        pv = ps_pj.tile([128, 512], F32, name=f"pv{s}_{t}{n}", tag="pj")
                        for k in range(KCH):
                            nc.tensor.matmul(
                                pv[:],
                                xT[k][:, t * 128:(t + 1) * 128],
                                wv_sb[:, k * E + n * 512: k * E + (n + 1) * 512],
                                start=(k == 0), stop=(k == KCH - 1))
                        hpc = E // NCH // D  # heads per chunk (8)
                        nc.vector.tensor_tensor(
                            vt3[:, n * hpc:(n + 1) * hpc, 0:D],
                            pv[:].rearrange("p (h d) -> p h d", d=D),
                            bvb[:].rearrange("p (h d) -> p h d", d=D)[:, n * hpc:(n + 1) * hpc, :],
                            op=mybir.AluOpType.add)
                    vsb.append(vt)

                for p in range(PAIRS):
                    # Q^T / K^T for this feature pair [128 feat, 512 tok]
                    qkt = {}
                    for nm, wsb, bt in (("q", wq_sb, bqt), ("k", wk_sb, bkt)):
                        pq = ps_pj.tile([128, 512], F32, name=f"pq{nm}{s}_{p}", tag="pj")
                        for k in range(KCH):
                            nc.tensor.matmul(
                                pq[:],
                                wsb[:, k * E + p * 128: k * E + (p + 1) * 128],
                                xT[k][:],
                                start=(k == 0), stop=(k == KCH - 1))
                        qt = qkp.tile([128, 512], F32R, name=f"{nm}T{s}_{p}", tag="qk")
                        nc.vector.tensor_scalar_add(qt[:], pq[:], bt[:, p:p + 1])
                        qkt[nm] = qt
                    QT, KT = qkt["q"], qkt["k"]

                    # S^T chunks + exp -> P^T, per head (row-tiled pairs)
                    PT = [[None] * TCH for _ in range(2)]
                    for c in range(TCH):
                        pse = ps_s.tile([128, 512], F32, name=f"pse{s}{p}{c}", tag="s")
                        pso = ps_s.tile([128, 512], F32, name=f"pso{s}{p}{c}", tag="s")
                        nc.tensor.matmul(
                            pse[:], KT[0:64, c * 128:(c + 1) * 128], QT[0:64, :],
                            start=True, stop=True, tile_position=(0, 0))
                        nc.tensor.matmul(
                            pso[:], KT[64:128, c * 128:(c + 1) * 128], QT[64:128, :],
                            start=True, stop=True, tile_position=(64, 0))
                        for hh, ps_t in ((0, pse), (1, pso)):
                            pt_t = ptp.tile([128, 512], F32R,
                                            name=f"pt{s}{p}{c}{hh}", tag="pt")
                            nc.scalar.activation(
                                pt_t[:], ps_t[:],
                                mybir.ActivationFunctionType.Exp, scale=float(SCALE))
                            PT[hh][c] = pt_t

                    # fused ctx^T + softmax-denominator per head:
                    # psum rows 0..63 = ctx^T, row 64 = l (ones column of V)
                    for hh in range(2):
                        h = 2 * p + hh
                        pc = ps_c.tile([65, 512], F32, name=f"pc{s}{p}{hh}", tag="c")
                        for c in range(TCH):
                            nc.tensor.matmul(
                                pc[:],
                                vsb[c][:, h * (D + 1):(h + 1) * (D + 1)],
                                PT[hh][c][:],
                                start=(c == 0), stop=(c == TCH - 1))
                        # 1/l = exp(-ln(l)) on ACT; broadcast over partitions
                        # via row-shift DMA + gpsimd partition_broadcast
                        nt = nrmp.tile([65, 1024], F32, name=f"nt{s}{p}{hh}", tag="nt")
                        nc.scalar.activation(nt[64:65, 0:512], pc[64:65, :],
                                             mybir.ActivationFunctionType.Ln)
                        nc.scalar.activation(nt[64:65, 512:1024], nt[64:65, 0:512],
                                             mybir.ActivationFunctionType.Exp,
                                             scale=-1.0)
                        nc.sync.dma_start(nt[0:1, 512:1024], nt[64:65, 512:1024])
                        nc.gpsimd.partition_broadcast(
                            nt[0:64, 0:512], nt[0:1, 512:1024])
                        st = stgp.tile([64, 512], F32R, name=f"st{s}{p}{hh}", tag="st")
                        nc.vector.tensor_tensor(st[:], pc[0:64, :], nt[0:64, 0:512],
                                                op=mybir.AluOpType.mult)
                        nc.sync.dma_start(
                            ctxT_dram[p * 128 + hh * 64: p * 128 + (hh + 1) * 64,
                                      s * C:(s + 1) * C],
                            st[:])

            # ---------------- phase B: output projection ----------------
            wo_sb = load_w("wo")
            for s in range(SEQS):
                ctxb = []
                for f in range(KCH):
                    cb = ptp.tile([128, 512], F32R, name=f"cb{s}_{f}", tag="pt")
                    nc.sync.dma_start(
                        cb[:], ctxT_dram[f * 128:(f + 1) * 128, s * C:(s + 1) * C])
                    ctxb.append(cb)
                for t in range(TCH):
                    for n in range(NCH):
                        po = ps_pj.tile([128, 512], F32, name=f"po{s}{t}{n}", tag="pj")
                        for k in range(KCH):
                            nc.tensor.matmul(
                                po[:],
                                ctxb[k][:, t * 128:(t + 1) * 128],
                                wo_sb[:, k * E + n * 512: k * E + (n + 1) * 512],
                                start=(k == 0), stop=(k == KCH - 1))
                        ob = osbp.tile([128, 512], F32, name=f"ob{s}{t}{n}", tag="ob")
                        nc.vector.tensor_tensor(
                            ob[:], po[:], bob[:, n * 512:(n + 1) * 512],
                            op=mybir.AluOpType.add)
                        nc.sync.dma_start(
                            os_d[s * C + t * 128: s * C + (t + 1) * 128,
                                 n * 512:(n + 1) * 512],
                            ob[:])

    nc.compile()
    return nc


_NC_CACHE = {}


def get_nc():
    if "nc" not in _NC_CACHE:
        _NC_CACHE["nc"] = build_nc()
    return _NC_CACHE["nc"]


def make_in_maps(x, wq, bq, wk, bk, wv, bv, wo, bo):
    x = np.asarray(x, dtype=np.float32)
    args = {n: np.asarray(v, dtype=np.float32)
            for n, v in (("wq", wq), ("bq", bq), ("wk", wk), ("bk", bk),
                         ("wv", wv), ("bv", bv), ("wo", wo), ("bo", bo))}
    xf = x.reshape(B * R, C, E)
    in_maps = []
    for c in range(NCORES):
        m = dict(args)
        m["xs"] = np.ascontiguousarray(
            xf[c * SEQS:(c + 1) * SEQS].reshape(SEQS * C, E))
        in_maps.append(m)
    return in_maps


def kernel(x, wq, bq, wk, bk, wv, bv, wo, bo):
    in_maps = make_in_maps(x, wq, bq, wk, bk, wv, bv, wo, bo)
    nc = get_nc()
    res = bass_utils.run_bass_kernel_spmd(
        nc, in_maps, core_ids=list(range(NCORES)))
    out = np.concatenate(
        [res.results[c]["os"].reshape(SEQS, C, E) for c in range(NCORES)], axis=0)
    return out.reshape(B, R, C, E).astype(np.float32)


# revision 9
# speedup vs baseline: 1.6824x; 1.3423x over previous
"""Multi-head attention Bass kernel for Trainium2, 8 NeuronCores.

Problem: B=2, R=16, C=512, E=1024, H=16 heads, D=64.
  q,k,v = x @ w{q,k,v} + b{q,k,v}  (per-head attention)  out = ctx @ wo + bo

Sharding: pure data parallel over the B*R = 32 independent (batch,row)
sequences -> 4 sequences of 512 tokens per core. No collectives.

Per-core plan (all matmuls float32r: ~1.5e-4 rel err, full PE rate at N>=512):
  - PE-transpose x tiles -> xT [feat, tok] (fp32, exact)
  - Q^T, K^T produced transposed:  psum[feat128, tok512] = wq_chunk.T @ xT
  - V produced natural:            psum[tok128, feat512] = xT_chunk.T @ wv
  - S^T[kj,qi] per head = (K^T chunk).T @ Q^T ; heads packed in K=64
    row-tile pairs (tile_position (0,0)/(64,0), separate psum banks)
  - P^T = exp(S^T/8) on ACT (no max subtraction: |logits| < ~3 for this
    data distribution, exp is safe in fp32)
  - ctx^T[d,qi] per head: 4-chunk psum accumulation of V_chunk.T @ P^T
  - l (softmax denom) broadcast over partitions free via all-ones
    stationary matmul; 1/l on DVE in fp32; normalize with tensor_tensor
  - ctx^T halves DMA'd to a DRAM bounce buffer (gives the odd head its
    partition shift for free), read back as O-projection stationaries
  - O = ctxT_chunk.T @ wo + bo -> DMA out

float32r hardware constraints honored: every producer of a matmul input
writes dtype float32r; dst base partition 0; no split-K accumulation
groups; no col-tiling.
"""

import numpy as np

import concourse.bacc as bacc
import concourse.mybir as mybir
import concourse.tile as tile
from concourse import bass_utils
from concourse.masks import make_identity

F32 = mybir.dt.float32
F32R = mybir.dt.float32r

# The kernel uses both Exp and Ln on ScalarE. Left alone, the table-load
# placement pass picks "exp_and_others" for Exp and "natural_log" for Ln,
# reloading the ACT tables (~2.7us) on every alternation. Restrict both
# functions to the one set that contains them together.
_orig_get_tables = bacc.get_activation_tables


def _combined_exp_ln_tables(arch):
    tabs = _orig_get_tables(arch)
    keep = "natural_log_exp_and_others"
    for name, fns in tabs.items():
        if name != keep:
            fns.discard(mybir.ActivationFunctionType.Exp)
            fns.discard(mybir.ActivationFunctionType.Ln)
    return tabs


bacc.get_activation_tables = _combined_exp_ln_tables

B, R, C, E, H = 2, 16, 512, 1024, 16
D = E // H            # 64
NCORES = 8
SEQS = (B * R) // NCORES   # 4 sequences per core
TCH = C // 128             # 4 token chunks per sequence
KCH = E // 128             # 8 contraction chunks
NCH = E // 512             # 2 output column chunks
PAIRS = H // 2             # 8 head pairs
SCALE = 1.0 / np.sqrt(D)   # folded into exp


def build_nc():
    nc = bacc.Bacc("TRN2", debug=False, num_devices=NCORES)

    xs_d = nc.dram_tensor("xs", [SEQS * C, E], F32, kind="ExternalInput").ap()
    w_d = {}
    for w in ("wq", "wk", "wv", "wo"):
        w_d[w] = nc.dram_tensor(w, [E, E], F32R, kind="ExternalInput").ap()
    b_d = {}
    for b in ("bq", "bk", "bv", "bo"):
        b_d[b] = nc.dram_tensor(b, [E], F32, kind="ExternalInput").ap()
    os_d = nc.dram_tensor("os", [SEQS * C, E], F32, kind="ExternalOutput").ap()

    with tile.TileContext(nc) as tc:
        with (
            tc.tile_pool(name="consts", bufs=1) as cpool,
            tc.tile_pool(name="wpool", bufs=3) as wpool,
            tc.tile_pool(name="xin", bufs=4) as xinp,
            tc.tile_pool(name="xT", bufs=9) as xTp,
            tc.tile_pool(name="vsb", bufs=4) as vp,
            tc.tile_pool(name="qk", bufs=3) as qkp,
            tc.tile_pool(name="pt", bufs=12) as ptp,
            tc.tile_pool(name="nrm", bufs=3) as nrmp,
            tc.tile_pool(name="stg", bufs=3) as stgp,
            tc.tile_pool(name="osb", bufs=2) as osbp,
            tc.tile_pool(name="dram", bufs=1, space="DRAM") as dpool,
            tc.tile_pool(name="ps_pj", bufs=2, space="PSUM") as ps_pj,
            tc.tile_pool(name="ps_s", bufs=4, space="PSUM") as ps_s,
            tc.tile_pool(name="ps_c", bufs=2, space="PSUM") as ps_c,
        ):
            # ---------------- constants ----------------
            ident = cpool.tile([128, 128], F32, name="ident")
            make_identity(nc, ident[:])
            ones_f = cpool.tile([128, 128], F32, name="ones_f")
            nc.vector.memset(ones_f[:], 1.0)
            onesr = cpool.tile([128, 128], F32R, name="onesr")
            nc.vector.tensor_copy(onesr[:], ones_f[:])

            # per-partition bias layouts: t[p, j] = b[j*128 + p]
            bqt = cpool.tile([128, KCH], F32, name="bqt")
            bkt = cpool.tile([128, KCH], F32, name="bkt")
            for name, t in (("bq", bqt), ("bk", bkt)):
                src = b_d[name].rearrange("(j p) -> p j", p=128)
                nc.sync.dma_start(t[:], src)

            # bv/bo broadcast to all 128 partitions (free-dim biases) via
            # all-ones outer product matmul
            bvb = cpool.tile([128, E], F32, name="bvb")
            bob = cpool.tile([128, E], F32, name="bob")
            # bv at partition 0, bo at partition 32 (matmul base_partition
            # must be in {0, 32, 64})
            brow = xinp.tile([33, E], F32, name="brow", tag="xin")
            nc.sync.dma_start(brow[0:1, :], b_d["bv"].rearrange("(o e) -> o e", o=1))
            nc.sync.dma_start(brow[32:33, :], b_d["bo"].rearrange("(o e) -> o e", o=1))
            browr = xinp.tile([33, E], F32R, name="browr", tag="xin")
            nc.vector.tensor_copy(browr[0:1, :], brow[0:1, :])
            nc.vector.tensor_copy(browr[32:33, :], brow[32:33, :])
            for j, dst in ((0, bvb), (32, bob)):
                for n in range(NCH):
                    pb = ps_pj.tile([128, 512], F32, name=f"pb{j}{n}", tag="pj")
                    nc.tensor.matmul(
                        pb[:], onesr[j:j + 1, :],
                        browr[j:j + 1, n * 512:(n + 1) * 512],
                        start=True, stop=True)
                    nc.vector.tensor_copy(dst[:, n * 512:(n + 1) * 512], pb[:])

            # weights: one [128, KCH*1024] tile per matrix; wo reuses a slot
            # after wq is dead (bufs=3)
            def load_w(name):
                t = wpool.tile([128, KCH * E], F32R, name=name, tag="w")
                for k in range(KCH):
                    nc.sync.dma_start(
                        t[:, k * E:(k + 1) * E], w_d[name][k * 128:(k + 1) * 128, :])
                return t

            wq_sb = load_w("wq")
            wk_sb = load_w("wk")
            wv_sb = load_w("wv")

            # ctx^T bounce buffer in DRAM
            ctxT_dram = dpool.tile([E, SEQS * C], F32R, name="ctxT_dram")

            # ---------------- phase A: projections + attention ----------
            for s in range(SEQS):
                # load x tiles [tok 128, E]
                xin = []
                for t in range(TCH):
                    xt = xinp.tile([128, E], F32, name=f"xin{s}_{t}", tag="xin")
                    nc.sync.dma_start(
                        xt[:], xs_d[s * C + t * 128: s * C + (t + 1) * 128, :])
                    xin.append(xt)

                # transpose -> xT[f] = [feat 128, tok 512] (f32r)
                xT = []
                for f in range(KCH):
                    ptr = ps_pj.tile([128, 512], F32, name=f"ptr{s}_{f}", tag="pj")
                    for t in range(TCH):
                        nc.tensor.transpose(
                            ptr[:, t * 128:(t + 1) * 128],
                            xin[t][:, f * 128:(f + 1) * 128], ident[:])
                    xf = xTp.tile([128, 512], F32R, name=f"xT{s}_{f}", tag="xT")
                    nc.vector.tensor_copy(xf[:], ptr[:])
                    xT.append(xf)

                # V projection: natural layout [tok 128, 16*(64+1)] with a
                # ones column appended per head (fused softmax-denominator)
                vsb = []
                for t in range(TCH):
                    vt = vp.tile([128, H * (D + 1)], F32R, name=f"v{s}_{t}", tag="v")
                    vt3 = vt[:].rearrange("p (h dd) -> p h dd", dd=D + 1)
                    nc.vector.tensor_copy(
                        vt3[:, :, D:D + 1],
                        ones_f[:].rearrange("p (a b) -> p a b", b=1)[:, 0:H, :])
                    for n in range(NCH):
                        pv = ps_pj.tile([128, 512], F32, name=f"pv{s}_{t}{n}", tag="pj")
                        for k in range(KCH):
                            nc.tensor.matmul(
                                pv[:],
                                xT[k][:, t * 128:(t + 1) * 128],
                                wv_sb[:, k * E + n * 512: k * E + (n + 1) * 512],
                                start=(k == 0), stop=(k == KCH - 1))
                        hpc = E // NCH // D  # heads per chunk (8)
                        nc.vector.tensor_tensor(
                            vt3[:, n * hpc:(n + 1) * hpc, 0:D],
                            pv[:].rearrange("p (h d) -> p h d", d=D),
                            bvb[:].rearrange("p (h d) -> p h d", d=D)[:, n * hpc:(n + 1) * hpc, :],
                            op=mybir.AluOpType.add)
                    vsb.append(vt)

                for p in range(PAIRS):
                    # Q^T / K^T for this feature pair [128 feat, 512 tok]
                    qkt = {}
                    for nm, wsb, bt in (("q", wq_sb, bqt), ("k", wk_sb, bkt)):
                        pq = ps_pj.tile([128, 512], F32, name=f"pq{nm}{s}_{p}", tag="pj")
                        for k in range(KCH):
                            nc.tensor.matmul(
                                pq[:],
                                wsb[:, k * E + p * 128: k * E + (p + 1) * 128],
                                xT[k][:],
                                start=(k == 0), stop=(k == KCH - 1))
                        qt = qkp.tile([128, 512], F32R, name=f"{nm}T{s}_{p}", tag="qk")
                        nc.vector.tensor_scalar_add(qt[:], pq[:], bt[:, p:p + 1])
                        qkt[nm] = qt
                    QT, KT = qkt["q"], qkt["k"]

                    # S^T chunks + exp -> P^T, per head (row-tiled pairs)
                    PT = [[None] * TCH for _ in range(2)]
                    for c in range(TCH):
                        pse = ps_s.tile([128, 512], F32, name=f"pse{s}{p}{c}", tag="s")
                        pso = ps_s.tile([128, 512], F32, name=f"pso{s}{p}{c}", tag="s")
                        nc.tensor.matmul(
                            pse[:], KT[0:64, c * 128:(c + 1) * 128], QT[0:64, :],
                            start=True, stop=True, tile_position=(0, 0))
                        nc.tensor.matmul(
                            pso[:], KT[64:128, c * 128:(c + 1) * 128], QT[64:128, :],
                            start=True, stop=True, tile_position=(64, 0))
                        for hh, ps_t in ((0, pse), (1, pso)):
                            pt_t = ptp.tile([128, 512], F32R,
                                            name=f"pt{s}{p}{c}{hh}", tag="pt")
                            nc.scalar.activation(
                                pt_t[:], ps_t[:],
                                mybir.ActivationFunctionType.Exp, scale=float(SCALE))
                            PT[hh][c] = pt_t

                    # fused ctx^T + softmax-denominator per head:
                    # psum rows 0..63 = ctx^T, row 64 = l (ones column of V)
                    for hh in range(2):
                        h = 2 * p + hh
                        pc = ps_c.tile([65, 512], F32, name=f"pc{s}{p}{hh}", tag="c")
                        for c in range(TCH):
                            nc.tensor.matmul(
                                pc[:],
                                vsb[c][:, h * (D + 1):(h + 1) * (D + 1)],
                                PT[hh][c][:],
                                start=(c == 0), stop=(c == TCH - 1))
                        # 1/l = exp(-ln(l)) on ACT; broadcast over partitions
                        # via row-shift DMA + gpsimd partition_broadcast
                        nt = nrmp.tile([65, 1024], F32, name=f"nt{s}{p}{hh}", tag="nt")
                        nc.scalar.activation(nt[64:65, 0:512], pc[64:65, :],
                                             mybir.ActivationFunctionType.Ln)
                        nc.scalar.activation(nt[64:65, 512:1024], nt[64:65, 0:512],
                                             mybir.ActivationFunctionType.Exp,
                                             scale=-1.0)
                        nc.sync.dma_start(nt[0:1, 512:1024], nt[64:65, 512:1024])
                        nc.gpsimd.partition_broadcast(
                            nt[0:64, 0:512], nt[0:1, 512:1024])
                        st = stgp.tile([64, 512], F32R, name=f"st{s}{p}{hh}", tag="st")
                        nc.vector.tensor_tensor(st[:], pc[0:64, :], nt[0:64, 0:512],
                                                op=mybir.AluOpType.mult)
                        nc.sync.dma_start(
                            ctxT_dram[p * 128 + hh * 64: p * 128 + (hh + 1) * 64,
                                      s * C:(s + 1) * C],
                            st[:])

            # ---------------- phase B: output projection ----------------
            wo_sb = load_w("wo")
            for s in range(SEQS):
                ctxb = []
                for f in range(KCH):
                    cb = ptp.tile([128, 512], F32R, name=f"cb{s}_{f}", tag="pt")
                    nc.sync.dma_start(
                        cb[:], ctxT_dram[f * 128:(f + 1) * 128, s * C:(s + 1) * C])
                    ctxb.append(cb)
                for t in range(TCH):
                    for n in range(NCH):
                        po = ps_pj.tile([128, 512], F32, name=f"po{s}{t}{n}", tag="pj")
                        for k in range(KCH):
                            nc.tensor.matmul(
                                po[:],
                                ctxb[k][:, t * 128:(t + 1) * 128],
                                wo_sb[:, k * E + n * 512: k * E + (n + 1) * 512],
                                start=(k == 0), stop=(k == KCH - 1))
                        ob = osbp.tile([128, 512], F32, name=f"ob{s}{t}{n}", tag="ob")
                        nc.vector.tensor_tensor(
                            ob[:], po[:], bob[:, n * 512:(n + 1) * 512],
                            op=mybir.AluOpType.add)
                        nc.sync.dma_start(
                            os_d[s * C + t * 128: s * C + (t + 1) * 128,
                                 n * 512:(n + 1) * 512],
                            ob[:])

    nc.compile()
    return nc


_NC_CACHE = {}


def get_nc():
    if "nc" not in _NC_CACHE:
        _NC_CACHE["nc"] = build_nc()
    return _NC_CACHE["nc"]


def make_in_maps(x, wq, bq, wk, bk, wv, bv, wo, bo):
    x = np.asarray(x, dtype=np.float32)
    args = {n: np.asarray(v, dtype=np.float32)
            for n, v in (("wq", wq), ("bq", bq), ("wk", wk), ("bk", bk),
                         ("wv", wv), ("bv", bv), ("wo", wo), ("bo", bo))}
    xf = x.reshape(B * R, C, E)
    in_maps = []
    for c in range(NCORES):
        m = dict(args)
        m["xs"] = np.ascontiguousarray(
            xf[c * SEQS:(c + 1) * SEQS].reshape(SEQS * C, E))
        in_maps.append(m)
    return in_maps


def kernel(x, wq, bq, wk, bk, wv, bv, wo, bo):
    in_maps = make_in_maps(x, wq, bq, wk, bk, wv, bv, wo, bo)
    nc = get_nc()
    res = bass_utils.run_bass_kernel_spmd(
        nc, in_maps, core_ids=list(range(NCORES)))
    out = np.concatenate(
        [res.results[c]["os"].reshape(SEQS, C, E) for c in range(NCORES)], axis=0)
    return out.reshape(B, R, C, E).astype(np.float32)


# revision 14
# speedup vs baseline: 1.9025x; 1.1308x over previous
"""Multi-head attention Bass kernel for Trainium2, 8 NeuronCores.

Problem: B=2, R=16, C=512, E=1024, H=16 heads, D=64.
  q,k,v = x @ w{q,k,v} + b{q,k,v}  (per-head attention)  out = ctx @ wo + bo

Sharding: pure data parallel over the B*R = 32 independent (batch,row)
sequences -> 4 sequences of 512 tokens per core. No collectives.

Per-core plan (all matmuls float32r: ~1.5e-4 rel err, full PE rate at N>=512):
  - PE-transpose x tiles -> xT [feat, tok] (fp32, exact)
  - Q^T, K^T produced transposed:  psum[feat128, tok512] = wq_chunk.T @ xT
  - V produced natural:            psum[tok128, feat512] = xT_chunk.T @ wv
  - S^T[kj,qi] per head = (K^T chunk).T @ Q^T ; heads packed in K=64
    row-tile pairs (tile_position (0,0)/(64,0), separate psum banks)
  - P^T = exp(S^T/8) on ACT (no max subtraction: |logits| < ~3 for this
    data distribution, exp is safe in fp32)
  - ctx^T[d,qi] per head: 4-chunk psum accumulation of V_chunk.T @ P^T
  - l (softmax denom) broadcast over partitions free via all-ones
    stationary matmul; 1/l on DVE in fp32; normalize with tensor_tensor
  - ctx^T halves DMA'd to a DRAM bounce buffer (gives the odd head its
    partition shift for free), read back as O-projection stationaries
  - O = ctxT_chunk.T @ wo + bo -> DMA out

float32r hardware constraints honored: every producer of a matmul input
writes dtype float32r; dst base partition 0; no split-K accumulation
groups; no col-tiling.
"""

import numpy as np

import concourse.bacc as bacc
import concourse.mybir as mybir
import concourse.tile as tile
from concourse import bass_utils
from concourse.masks import make_identity

F32 = mybir.dt.float32
F32R = mybir.dt.float32r

# The kernel uses both Exp and Ln on ScalarE. Left alone, the table-load
# placement pass picks "exp_and_others" for Exp and "natural_log" for Ln,
# reloading the ACT tables (~2.7us) on every alternation. Restrict both
# functions to the one set that contains them together.
_orig_get_tables = bacc.get_activation_tables


def _combined_exp_ln_tables(arch):
    tabs = _orig_get_tables(arch)
    keep = "natural_log_exp_and_others"
    for name, fns in tabs.items():
        if name != keep:
            fns.discard(mybir.ActivationFunctionType.Exp)
            fns.discard(mybir.ActivationFunctionType.Ln)
    return tabs


bacc.get_activation_tables = _combined_exp_ln_tables

B, R, C, E, H = 2, 16, 512, 1024, 16
D = E // H            # 64
NCORES = 8
SEQS = (B * R) // NCORES   # 4 sequences per core
TCH = C // 128             # 4 token chunks per sequence
KCH = E // 128             # 8 contraction chunks
NCH = E // 512             # 2 output column chunks
PAIRS = H // 2             # 8 head pairs
SCALE = 1.0 / np.sqrt(D)   # folded into exp


def build_nc():
    nc = bacc.Bacc("TRN2", debug=False, num_devices=NCORES)

    xs_d = nc.dram_tensor("xs", [SEQS * C, E], F32, kind="ExternalInput").ap()
    w_d = {}
    for w in ("wq", "wk", "wv", "wo"):
        w_d[w] = nc.dram_tensor(w, [E, E], F32R, kind="ExternalInput").ap()
    b_d = {}
    for b in ("bq", "bk", "bv", "bo"):
        b_d[b] = nc.dram_tensor(b, [E], F32, kind="ExternalInput").ap()
    os_d = nc.dram_tensor("os", [SEQS * C, E], F32, kind="ExternalOutput").ap()

    with tile.TileContext(nc) as tc:
        with (
            tc.tile_pool(name="consts", bufs=1) as cpool,
            tc.tile_pool(name="wpool", bufs=3) as wpool,
            tc.tile_pool(name="xin", bufs=4) as xinp,
            tc.tile_pool(name="xT", bufs=9) as xTp,
            tc.tile_pool(name="vsb", bufs=4) as vp,
            tc.tile_pool(name="qk", bufs=3) as qkp,
            tc.tile_pool(name="pt", bufs=6) as ptp,
            tc.tile_pool(name="nrm", bufs=3) as nrmp,
            tc.tile_pool(name="stg", bufs=3) as stgp,
            tc.tile_pool(name="osb", bufs=2) as osbp,
            tc.tile_pool(name="dram", bufs=1, space="DRAM") as dpool,
            tc.tile_pool(name="ps_pj", bufs=2, space="PSUM") as ps_pj,
            tc.tile_pool(name="ps_s", bufs=2, space="PSUM") as ps_s,
            tc.tile_pool(name="ps_c", bufs=2, space="PSUM") as ps_c,
        ):
            # ---------------- constants ----------------
            ident = cpool.tile([128, 128], F32, name="ident")
            make_identity(nc, ident[:])
            ones_f = cpool.tile([128, 128], F32, name="ones_f")
            nc.vector.memset(ones_f[:], 1.0)
            onesr = cpool.tile([128, 128], F32R, name="onesr")
            nc.vector.tensor_copy(onesr[:], ones_f[:])

            # per-partition bias layouts: t[p, j] = b[j*128 + p]
            bqt = cpool.tile([128, KCH], F32, name="bqt")
            bkt = cpool.tile([128, KCH], F32, name="bkt")
            for name, t in (("bq", bqt), ("bk", bkt)):
                src = b_d[name].rearrange("(j p) -> p j", p=128)
                nc.sync.dma_start(t[:], src)

            # bv/bo broadcast to all 128 partitions (free-dim biases) via
            # all-ones outer product matmul
            bvb = cpool.tile([128, E], F32, name="bvb")
            bob = cpool.tile([128, E], F32, name="bob")
            # bv at partition 0, bo at partition 32 (matmul base_partition
            # must be in {0, 32, 64})
            brow = xinp.tile([33, E], F32, name="brow", tag="xin")
            nc.sync.dma_start(brow[0:1, :], b_d["bv"].rearrange("(o e) -> o e", o=1))
            nc.sync.dma_start(brow[32:33, :], b_d["bo"].rearrange("(o e) -> o e", o=1))
            browr = xinp.tile([33, E], F32R, name="browr", tag="xin")
            nc.vector.tensor_copy(browr[0:1, :], brow[0:1, :])
            nc.vector.tensor_copy(browr[32:33, :], brow[32:33, :])
            for j, dst in ((0, bvb), (32, bob)):
                for n in range(NCH):
                    pb = ps_pj.tile([128, 512], F32, name=f"pb{j}{n}", tag="pj")
                    nc.tensor.matmul(
                        pb[:], onesr[j:j + 1, :],
                        browr[j:j + 1, n * 512:(n + 1) * 512],
                        start=True, stop=True)
                    nc.vector.tensor_copy(dst[:, n * 512:(n + 1) * 512], pb[:])

            # weights: one [128, KCH*1024] tile per matrix; wo reuses a slot
            # after wq is dead (bufs=3)
            def load_w(name):
                t = wpool.tile([128, KCH * E], F32R, name=name, tag="w")
                for k in range(KCH):
                    nc.sync.dma_start(
                        t[:, k * E:(k + 1) * E], w_d[name][k * 128:(k + 1) * 128, :])
                return t

            # prefetch seq-0 x tiles before the 12MB of weight DMAs so the
            # transposes start immediately
            xin_pre = []
            for t in range(TCH):
                xt = xinp.tile([128, E], F32, name=f"xin0_{t}", tag="xin")
                nc.sync.dma_start(xt[:], xs_d[t * 128:(t + 1) * 128, :])
                xin_pre.append(xt)

            wv_sb = load_w("wv")
            wq_sb = load_w("wq")
            wk_sb = load_w("wk")

            # ctx^T bounce buffer in DRAM
            ctxT_dram = dpool.tile([E, SEQS * C], F32R, name="ctxT_dram")

            # ---------------- phase A: projections + attention ----------
            for s in range(SEQS):
                # load x tiles [tok 128, E]
                if s == 0:
                    xin = xin_pre
                else:
                    xin = []
                    for t in range(TCH):
                        xt = xinp.tile([128, E], F32, name=f"xin{s}_{t}", tag="xin")
                        nc.sync.dma_start(
                            xt[:], xs_d[s * C + t * 128: s * C + (t + 1) * 128, :])
                        xin.append(xt)

                # transpose -> xT[f] = [feat 128, tok 512] (f32r)
                xT = []
                for f in range(KCH):
                    ptr = ps_pj.tile([128, 512], F32, name=f"ptr{s}_{f}", tag="pj")
                    for t in range(TCH):
                        nc.tensor.transpose(
                            ptr[:, t * 128:(t + 1) * 128],
                            xin[t][:, f * 128:(f + 1) * 128], ident[:])
                    xf = xTp.tile([128, 512], F32R, name=f"xT{s}_{f}", tag="xT")
                    nc.vector.tensor_copy(xf[:], ptr[:])
                    xT.append(xf)

                # V projection: natural layout [tok 128, 16*(64+1)] with a
                # ones column appended per head (fused softmax-denominator)
                vsb = []
                for t in range(TCH):
                    vt = vp.tile([128, H * (D + 1)], F32R, name=f"v{s}_{t}", tag="v")
                    vt3 = vt[:].rearrange("p (h dd) -> p h dd", dd=D + 1)
                    nc.vector.tensor_copy(
                        vt3[:, :, D:D + 1],
                        ones_f[:].rearrange("p (a b) -> p a b", b=1)[:, 0:H, :])
                    for n in range(NCH):
                        pv = ps_pj.tile([128, 512], F32, name=f"pv{s}_{t}{n}", tag="pj")
                        for k in range(KCH):
                            nc.tensor.matmul(
                                pv[:],
                                xT[k][:, t * 128:(t + 1) * 128],
                                wv_sb[:, k * E + n * 512: k * E + (n + 1) * 512],
                                start=(k == 0), stop=(k == KCH - 1))
                        hpc = E // NCH // D  # heads per chunk (8)
                        nc.vector.tensor_tensor(
                            vt3[:, n * hpc:(n + 1) * hpc, 0:D],
                            pv[:].rearrange("p (h d) -> p h d", d=D),
                            bvb[:].rearrange("p (h d) -> p h d", d=D)[:, n * hpc:(n + 1) * hpc, :],
                            op=mybir.AluOpType.add)
                    vsb.append(vt)

                for p in range(PAIRS):
                    # Q^T / K^T for this feature pair [128 feat, 512 tok]
                    qkt = {}
                    for nm, wsb, bt in (("q", wq_sb, bqt), ("k", wk_sb, bkt)):
                        pq = ps_pj.tile([128, 512], F32, name=f"pq{nm}{s}_{p}", tag="pj")
                        for k in range(KCH):
                            nc.tensor.matmul(
                                pq[:],
                                wsb[:, k * E + p * 128: k * E + (p + 1) * 128],
                                xT[k][:],
                                start=(k == 0), stop=(k == KCH - 1))
                        qt = qkp.tile([128, 512], F32R, name=f"{nm}T{s}_{p}", tag="qk")
                        nc.vector.tensor_scalar_add(qt[:], pq[:], bt[:, p:p + 1])
                        qkt[nm] = qt
                    QT, KT = qkt["q"], qkt["k"]

                    # S^T chunks + exp -> P^T, per head (row-tiled pairs).
                    # Two kj-chunks share one 2-bank psum tile so each exp
                    # covers [128,1024].
                    PT2 = [[None, None] for _ in range(2)]
                    for cp in range(TCH // 2):
                        pse = ps_s.tile([128, 1024], F32, name=f"pse{s}{p}{cp}", tag="s")
                        pso = ps_s.tile([128, 1024], F32, name=f"pso{s}{p}{cp}", tag="s")
                        for ci in range(2):
                            c = 2 * cp + ci
                            nc.tensor.matmul(
                                pse[:, ci * 512:(ci + 1) * 512],
                                KT[0:64, c * 128:(c + 1) * 128], QT[0:64, :],
                                start=True, stop=True, tile_position=(0, 0))
                            nc.tensor.matmul(
                                pso[:, ci * 512:(ci + 1) * 512],
                                KT[64:128, c * 128:(c + 1) * 128], QT[64:128, :],
                                start=True, stop=True, tile_position=(64, 0))
                        for hh, ps_t in ((0, pse), (1, pso)):
                            pt_t = ptp.tile([128, 1024], F32R,
                                            name=f"pt{s}{p}{cp}{hh}", tag="pt")
                            nc.scalar.activation(
                                pt_t[:], ps_t[:],
                                mybir.ActivationFunctionType.Exp, scale=float(SCALE))
                            PT2[hh][cp] = pt_t

                    # fused ctx^T + softmax-denominator per head:
                    # psum rows 0..63 = ctx^T, row 64 = l (ones column of V)
                    for hh in range(2):
                        h = 2 * p + hh
                        pc = ps_c.tile([65, 512], F32, name=f"pc{s}{p}{hh}", tag="c")
                        for c in range(TCH):
                            nc.tensor.matmul(
                                pc[:],
                                vsb[c][:, h * (D + 1):(h + 1) * (D + 1)],
                                PT2[hh][c // 2][:, (c % 2) * 512:(c % 2 + 1) * 512],
                                start=(c == 0), stop=(c == TCH - 1))
                        # 1/l = exp(-ln(l)) on ACT (both functions live in the
                        # natural_log_exp_and_others table set -> no reloads);
                        # broadcast over partitions via row-shift DMA +
                        # gpsimd partition_broadcast
                        nt = nrmp.tile([65, 1024], F32, name=f"nt{s}{p}{hh}", tag="nt")
                        nc.scalar.activation(nt[64:65, 0:512], pc[64:65, :],
                                             mybir.ActivationFunctionType.Ln)
                        nc.scalar.activation(nt[64:65, 512:1024], nt[64:65, 0:512],
                                             mybir.ActivationFunctionType.Exp,
                                             scale=-1.0)
                        nc.sync.dma_start(nt[0:1, 512:1024], nt[64:65, 512:1024])
                        nc.gpsimd.partition_broadcast(
                            nt[0:64, 0:512], nt[0:1, 512:1024])
                        st = stgp.tile([64, 512], F32R, name=f"st{s}{p}{hh}", tag="st")
                        nc.vector.tensor_tensor(st[:], pc[0:64, :], nt[0:64, 0:512],
                                                op=mybir.AluOpType.mult)
                        nc.sync.dma_start(
                            ctxT_dram[p * 128 + hh * 64: p * 128 + (hh + 1) * 64,
                                      s * C:(s + 1) * C],
                            st[:])

            # ---------------- phase B: output projection ----------------
            wo_sb = load_w("wo")
            for s in range(SEQS):
                ctxb = []
                for fp in range(KCH // 2):
                    cb = ptp.tile([128, 1024], F32R, name=f"cb{s}_{fp}", tag="pt")
                    for fi in range(2):
                        f = 2 * fp + fi
                        nc.sync.dma_start(
                            cb[:, fi * 512:(fi + 1) * 512],
                            ctxT_dram[f * 128:(f + 1) * 128, s * C:(s + 1) * C])
                    ctxb.append(cb)
                for t in range(TCH):
                    for n in range(NCH):
                        po = ps_pj.tile([128, 512], F32, name=f"po{s}{t}{n}", tag="pj")
                        for k in range(KCH):
                            nc.tensor.matmul(
                                po[:],
                                ctxb[k // 2][:, (k % 2) * 512 + t * 128:
                                             (k % 2) * 512 + (t + 1) * 128],
                                wo_sb[:, k * E + n * 512: k * E + (n + 1) * 512],
                                start=(k == 0), stop=(k == KCH - 1))
                        ob = osbp.tile([128, 512], F32, name=f"ob{s}{t}{n}", tag="ob")
                        nc.vector.tensor_tensor(
                            ob[:], po[:], bob[:, n * 512:(n + 1) * 512],
                            op=mybir.AluOpType.add)
                        nc.sync.dma_start(
                            os_d[s * C + t * 128: s * C + (t + 1) * 128,
                                 n * 512:(n + 1) * 512],
                            ob[:])

    nc.compile()
    return nc


_NC_CACHE = {}


def get_nc():
    if "nc" not in _NC_CACHE:
        _NC_CACHE["nc"] = build_nc()
    return _NC_CACHE["nc"]


def make_in_maps(x, wq, bq, wk, bk, wv, bv, wo, bo):
    x = np.asarray(x, dtype=np.float32)
    args = {n: np.asarray(v, dtype=np.float32)
            for n, v in (("wq", wq), ("bq", bq), ("wk", wk), ("bk", bk),
                         ("wv", wv), ("bv", bv), ("wo", wo), ("bo", bo))}
    xf = x.reshape(B * R, C, E)
    in_maps = []
    for c in range(NCORES):
        m = dict(args)
        m["xs"] = np.ascontiguousarray(
            xf[c * SEQS:(c + 1) * SEQS].reshape(SEQS * C, E))
        in_maps.append(m)
    return in_maps


def kernel(x, wq, bq, wk, bk, wv, bv, wo, bo):
    in_maps = make_in_maps(x, wq, bq, wk, bk, wv, bv, wo, bo)
    nc = get_nc()
    res = bass_utils.run_bass_kernel_spmd(
        nc, in_maps, core_ids=list(range(NCORES)))
    out = np.concatenate(
        [res.results[c]["os"].reshape(SEQS, C, E) for c in range(NCORES)], axis=0)
    return out.reshape(B, R, C, E).astype(np.float32)
